# revision 59
# baseline (speedup 1.0000x reference)
"""Causal self-attention on 8 Trainium2 NeuronCores (Bass/Tile).

Problem: y = CausalSelfAttention(x; Wq, Wk, Wv, Wo) with
B=4, S=2048, E=1024, H=16 heads of 64, fp32 inputs/outputs.

Sharding (tensor-parallel x data-parallel): core c of 8 handles batch
b = c//2 and head-group g = c%2 (8 of 16 heads). Each core receives
x[b] [S, E], the head-group's columns of Wq/Wk/Wv [E, 512] and rows of
Wo [512, E], and produces a partial output projection [S, E]. The host
sums the two partials per batch.

Per-core dataflow (attention-path operands bf16, accumulation fp32):
  x arrives pre-transposed from the host (xt [E, S]) so every DMA is a
    plain contiguous 2D tile load on one queue, ordered by first use
    (the on-device XBAR transpose is a serial ~40us resource that
    otherwise gates the whole pipeline)
  qT/kT [512, S] = W.T @ x.T and v [S, 512] = x @ Wv, plain
    PSUM-accumulated matmul chains (kt-ordered so they pipeline with
    the arriving x tiles)
  per head-pair t, q-chunk of 512, k-tile of 128:
    ST [128, 2x512] = K @ Q.T   (two heads row-packed into disjoint
                                 64-row PE tile groups, concurrent)
    PT = exp(ST/8)              (one wide ACTIVATE per pair; causal mask
                                 = triangular-mask multiply on the
                                 diagonal subblock, both heads in one
                                 strided DVE op)
    AV [128, 512] += [V_h0|V_h1] col-packed PV pair (M=64+64, own rhs
                                 streams); SM [128, 512] += ones-matmul
                                 pair producing broadcast softmax sums
                                 (same col-packed shape, no extra pass
                                 for the bc broadcast)
  attT = AV * reciprocal(SM)    (two DVE ops per (pair, q-chunk))
  out = sum_t attT_t.T @ Wo_t accumulated over all 4 pairs in PSUM,
    evacuated once per output tile, DMA'd out.

Scheduling: the attention inner loop is balanced ~1:1 between the PE
(QK + PV + sums, ~1.0us/k-tile-pair) and the scalar engine's EXP
stream (~1.9us/pair, ~150us total), so projection work is diced into
8-matmul bursts and popped one-per-pair between the QK and PV groups
of each attention unit; the t=3 units interleave the output projection
the same way.  PSUM layout: QK double-buffer 4 banks, projection ring
2 banks (its own pool, so fillers never serialize behind the QK->EXP
pipeline), AV 1, SM 1.

No device collectives; the host slices inputs and sums partials.
"""

import numpy as np

import concourse.bass as bass
import concourse.mybir as mybir
from concourse import bacc
from concourse.tile import TileContext

FP = mybir.dt.float32
BF = mybir.dt.bfloat16
P = 128


def build(S=2048, E=1024, HPC=8, DH=64, NQ=512):
    GD = HPC * DH          # 512 head dims per core
    KT_E = E // P          # 8 contraction tiles over E
    ST_S = S // P          # 16 sequence tiles
    QC = S // NQ           # 4 q-chunks
    DT = GD // P           # 4 head pairs
    QSUB = NQ // P         # 4 k-tiles per q-chunk step

    assert DH == 64 and NQ % P == 0 and S % NQ == 0 and E % P == 0

    nc = bacc.Bacc(None, target_bir_lowering=False)
    xt_d = nc.dram_tensor("xt", [E, S], BF, kind="ExternalInput")
    wq_d = nc.dram_tensor("wq", [E, GD], BF, kind="ExternalInput")
    wk_d = nc.dram_tensor("wk", [E, GD], BF, kind="ExternalInput")
    wv_d = nc.dram_tensor("wv", [E, GD], BF, kind="ExternalInput")
    wo_d = nc.dram_tensor("wo", [GD, E], BF, kind="ExternalInput")
    out_d = nc.dram_tensor("out", [S, E], FP, kind="ExternalOutput")

    with TileContext(nc) as tc:
        with (
            tc.tile_pool(name="consts", bufs=1) as consts,
            tc.tile_pool(name="data", bufs=1) as data,
            tc.tile_pool(name="xT", bufs=1) as xT_pool,
            tc.tile_pool(name="wbuf", bufs=1) as wbuf,
            tc.tile_pool(name="pt_sb", bufs=8) as pt_pool,
            tc.tile_pool(name="rec_sb", bufs=2) as rec_pool,
            tc.tile_pool(name="po_sb", bufs=2) as posb_pool,
            tc.tile_pool(name="st_psum", bufs=2, space="PSUM") as st_pool,
            tc.tile_pool(name="pj_psum", bufs=2, space="PSUM") as pj_pool,
            tc.tile_pool(name="av_psum", bufs=1, space="PSUM") as av_pool,
            tc.tile_pool(name="sm_psum", bufs=1, space="PSUM") as sm_pool,
        ):
            # ---- constants -------------------------------------------------
            ones64 = consts.tile([P, 64], BF)
            nc.vector.memset(ones64[:], 1.0)
            # upper-triangular-inclusive multiplicative mask (valid k <= q),
            # replicated side by side for the two packed heads
            ut = consts.tile([P, P], BF)
            nc.gpsimd.memset(ut[:], 0.0)
            nc.gpsimd.affine_select(
                out=ut[:], in_=ut[:],
                compare_op=mybir.AluOpType.is_gt, fill=1.0,
                base=0, pattern=[[-1, P]], channel_multiplier=1,
            )
            ut2 = consts.tile([P, 2 * P], BF)
            nc.vector.tensor_copy(ut2[:, 0:P], ut[:])
            nc.vector.tensor_copy(ut2[:, P : 2 * P], ut[:])
            # preload the EXP table set while DMAs are in flight
            dummy_in = consts.tile([1, 1], FP)
            nc.vector.memset(dummy_in[:], 0.0)
            dummy_out = consts.tile([1, 1], BF)
            nc.scalar.activation(
                dummy_out[:], dummy_in[:], mybir.ActivationFunctionType.Exp,
                scale=1.0,
            )

            # ---- persistent SBUF data -------------------------------------
            kT = [data.tile([P, S], BF, tag=f"kT{t}", name=f"kT{t}") for t in range(DT)]
            qT = [data.tile([P, S], BF, tag=f"qT{t}", name=f"qT{t}") for t in range(DT)]
            v = [data.tile([P, GD], BF, tag=f"v{st}", name=f"v{st}") for st in range(ST_S)]
            attT = [data.tile([P, S], BF, tag=f"attT{t}", name=f"attT{t}") for t in range(DT)]
            xTc = [
                [xT_pool.tile([P, NQ], BF, tag=f"xT{et}_{sc}", name=f"xT{et}_{sc}")
                 for sc in range(QC)]
                for et in range(KT_E)
            ]
            wk_all = [wbuf.tile([P, KT_E * P], BF, tag=f"wk{mt}", name=f"wk{mt}") for mt in range(DT)]
            wq_all = [wbuf.tile([P, KT_E * P], BF, tag=f"wq{mt}", name=f"wq{mt}") for mt in range(DT)]
            wv_all = wbuf.tile([P, KT_E * GD], BF, tag="wv", name="wv")
            wo_all = wbuf.tile([P, DT * E], BF, tag="wo", name="wo")

            # One serial DMA queue, transfers ordered by first use.  x
            # arrives pre-transposed from the host, so every load is a
            # plain contiguous 2D tile descriptor (no XBAR serialization).
            def emit_xt(sc):
                for et in range(KT_E):
                    nc.sync.dma_start(
                        out=xTc[et][sc][:],
                        in_=xt_d[et * P : (et + 1) * P, sc * NQ : (sc + 1) * NQ],
                    )

            def emit_wkq(mt):
                for w_d, all_t in ((wk_d, wk_all), (wq_d, wq_all)):
                    for kt in range(KT_E):
                        nc.sync.dma_start(
                            out=all_t[mt][:, kt * P : (kt + 1) * P],
                            in_=w_d[kt * P : (kt + 1) * P, mt * P : (mt + 1) * P],
                        )

            # chunk0 x / k0 / q0 / wv interleaved per-kt so every chunk-0
            # projection chain (k, q, and v) starts ~1us in and pipelines
            # kt-by-kt with the DMA stream
            for kt in range(KT_E):
                nc.sync.dma_start(
                    out=xTc[kt][0][:], in_=xt_d[kt * P : (kt + 1) * P, 0:NQ]
                )
                nc.sync.dma_start(
                    out=wk_all[0][:, kt * P : (kt + 1) * P],
                    in_=wk_d[kt * P : (kt + 1) * P, 0:P],
                )
                nc.sync.dma_start(
                    out=wq_all[0][:, kt * P : (kt + 1) * P],
                    in_=wq_d[kt * P : (kt + 1) * P, 0:P],
                )
                nc.sync.dma_start(
                    out=wv_all[:, kt * GD : (kt + 1) * GD],
                    in_=wv_d[kt * P : (kt + 1) * P, :],
                )
            emit_xt(1)
            emit_wkq(1)
            emit_xt(2)
            emit_wkq(2)
            for t in range(DT):
                nc.sync.dma_start(
                    out=wo_all[:, t * E : (t + 1) * E],
                    in_=wo_d[t * P : (t + 1) * P, :],
                )
            emit_xt(3)
            emit_wkq(3)

            def kq_operands(mt, nsc, which, kt):
                w_all = (wk_all, wq_all)[which][mt]
                return w_all[:, kt * P : (kt + 1) * P], xTc[kt][nsc][:]

            def v_operands(st_i, kt):
                sc, r = divmod(st_i * P, NQ)
                return (xTc[kt][sc][:, r : r + P],
                        wv_all[:, kt * GD : (kt + 1) * GD])

            def proj_halves(operands, cast_out, name):
                """A projection chain as two ~1us filler closures: the first
                opens the PSUM accumulation (kt 0-3), the second finishes it
                and casts.  Halves are always consumed in order (FIFO)."""
                box = {}
                half_kt = KT_E // 2

                def first():
                    ps = pj_pool.tile([P, NQ], FP, tag="pj", name=name)
                    box["ps"] = ps
                    for kt in range(half_kt):
                        lhsT, rhs = operands(kt)
                        nc.tensor.matmul(ps[:, 0:NQ], lhsT=lhsT, rhs=rhs,
                                         start=(kt == 0), stop=False)

                def second():
                    ps = box["ps"]
                    for kt in range(half_kt, KT_E):
                        lhsT, rhs = operands(kt)
                        nc.tensor.matmul(ps[:, 0:NQ], lhsT=lhsT, rhs=rhs,
                                         start=False, stop=(kt == KT_E - 1))
                    nc.vector.tensor_copy(cast_out, ps[:, 0:NQ])

                return [first, second]

            def kq_halves(mt, nsc, which):
                dstT = (kT, qT)[which]
                return proj_halves(
                    lambda kt, m=mt, n=nsc, w=which: kq_operands(m, n, w, kt),
                    dstT[mt][:, nsc * NQ : (nsc + 1) * NQ],
                    f"pj{which}_{mt}_{nsc}",
                )

            def v_halves(st_i):
                return proj_halves(
                    lambda kt, s=st_i: v_operands(s, kt),
                    v[st_i][:], f"pv{st_i}",
                )

            def emit_proj_kq(mt, nsc, which):
                for f in kq_halves(mt, nsc, which):
                    f()

            def emit_proj_v(st_i):
                for f in v_halves(st_i):
                    f()

            def emit_outproj(st, nj):
                po = pj_pool.tile([P, NQ], FP, tag="pj", name=f"po{st}_{nj}")
                for t in range(DT):
                    nc.tensor.matmul(
                        po[:, 0:NQ],
                        lhsT=attT[t][:, st * P : (st + 1) * P],
                        rhs=wo_all[:, t * E + nj * NQ : t * E + (nj + 1) * NQ],
                        start=(t == 0), stop=(t == DT - 1),
                    )
                posb = posb_pool.tile([P, NQ], FP, tag="posb")
                nc.vector.tensor_copy(posb[:], po[:])
                nc.sync.dma_start(
                    out=out_d[st * P : (st + 1) * P, nj * NQ : (nj + 1) * NQ],
                    in_=posb[:],
                )

            def attn_unit(t, qj, fillers):
                n_tiles = QSUB * qj + QSUB
                kmax = n_tiles - 1
                av = av_pool.tile([P, NQ], FP, tag="av", name=f"av{t}_{qj}")
                sm = sm_pool.tile([P, NQ], FP, tag="sm", name=f"sm{t}_{qj}")

                def qk(ki):
                    stp = st_pool.tile([P, 2 * NQ], FP, tag="st")
                    d = ki - QSUB * qj
                    off = P * d if d > 0 else 0
                    for half in range(2):
                        pr = 64 * half
                        nc.tensor.matmul(
                            stp[:, half * NQ + off : (half + 1) * NQ],
                            lhsT=kT[t][pr : pr + 64, ki * P : (ki + 1) * P],
                            rhs=qT[t][pr : pr + 64, qj * NQ + off : (qj + 1) * NQ],
                            start=True, stop=True,
                        )
                    return stp, off, d

                def exp_mask(stp, off, d):
                    pt = pt_pool.tile([P, 2 * NQ], BF, tag="pt")
                    if off == 0:
                        nc.scalar.activation(
                            pt[:, 0 : 2 * NQ], stp[:, 0 : 2 * NQ],
                            mybir.ActivationFunctionType.Exp, scale=0.125,
                        )
                    else:
                        # one ACTIVATE over both heads' valid spans via a
                        # strided AP; dead cols are never read downstream
                        pt2 = pt.rearrange("p (k c) -> p k c", c=NQ)
                        st2 = stp.rearrange("p (k c) -> p k c", c=NQ)
                        nc.scalar.activation(
                            pt2[:, :, off:NQ], st2[:, :, off:NQ],
                            mybir.ActivationFunctionType.Exp, scale=0.125,
                        )
                    if d >= 0:
                        # causal mask on the diagonal subblock, both heads in
                        # one strided op
                        pt3 = pt.rearrange("p (k c) -> p k c", c=NQ)
                        ut3 = ut2.rearrange("p (k c) -> p k c", c=P)
                        nc.vector.tensor_tensor(
                            pt3[:, :, off : off + P], pt3[:, :, off : off + P],
                            ut3[:], mybir.AluOpType.mult,
                        )
                    return pt

                def pv_sums(pt, off, ki):
                    st_f, sp_f = (ki == 0), (ki == kmax)
                    for half in range(2):
                        h = 2 * t + half
                        nc.tensor.matmul(
                            av[64 * half : 64 * half + 64, off:NQ],
                            lhsT=v[ki][:, h * DH : h * DH + DH],
                            rhs=pt[:, half * NQ + off : (half + 1) * NQ],
                            start=st_f, stop=sp_f, skip_group_check=True,
                        )
                        nc.tensor.matmul(
                            sm[64 * half : 64 * half + 64, off:NQ],
                            lhsT=ones64[:],
                            rhs=pt[:, half * NQ + off : (half + 1) * NQ],
                            start=st_f, stop=sp_f, skip_group_check=True,
                        )

                # ki-pairs keep same-shape instruction streaks on the PE;
                # one projection burst rides between consecutive pairs
                for kp in range(n_tiles // 2):
                    kis = (2 * kp, 2 * kp + 1)
                    sts = [qk(ki) for ki in kis]
                    pts = [exp_mask(stp, off, d) for stp, off, d in sts]
                    # filler rides between QK and PV in the PE stream, so
                    # the PE chews it while the scalar engine runs the EXPs
                    if fillers:
                        fillers.pop(0)()
                    for ki, pt, (stp, off, d) in zip(kis, pts, sts):
                        pv_sums(pt, off, ki)

                while fillers:
                    fillers.pop(0)()

                rec = rec_pool.tile([P, NQ], FP, tag="rec")
                nc.vector.reciprocal_approx_fast(rec[:], sm[:])
                nc.vector.tensor_tensor(
                    attT[t][:, qj * NQ : (qj + 1) * NQ], av[:], rec[:],
                    mybir.AluOpType.mult,
                )

            # ---- main pipeline --------------------------------------------
            def chunk_group(nsc):
                g = [lambda n=nsc: emit_proj_kq(0, n, 0),
                     lambda n=nsc: emit_proj_kq(0, n, 1)]
                g += [lambda s=st_i: emit_proj_v(s)
                      for st_i in range(4 * nsc, 4 * nsc + 4)]
                return g

            # chunk 0 must fully precede the first attention unit
            for f in chunk_group(0):
                f()

            pending = []  # (needed_before_key, closure), key = (t, qj)
            for nsc in range(1, QC):
                pending += [((0, nsc), f) for f in chunk_group(nsc)]
            for mt in range(1, DT):
                for nsc in range(QC):
                    for which in (0, 1):
                        pending.append((
                            (mt, 0 if mt > 1 else nsc),
                            lambda m=mt, n=nsc, w=which: emit_proj_kq(m, n, w),
                        ))

            for t in range(DT):
                for qj in range(QC):
                    while pending and pending[0][0] <= (t, qj):
                        pending.pop(0)[1]()
                    fillers = []
                    if pending:
                        # t0 capped at 3 so a few projection bursts survive
                        # to feed the fillerless t3-qj0 unit
                        take = min(len(pending), (QSUB * qj + QSUB) >> 1,
                                   3 if t == 0 else 4)
                        fillers = [f for _, f in pending[:take]]
                        del pending[:take]
                    if t == DT - 1 and qj > 0:
                        # last pair: interleave the output projection of the
                        # previous q-chunk's s-tiles
                        fillers += [
                            lambda s=st, n=nj: emit_outproj(s, n)
                            for st in range(4 * (qj - 1), 4 * qj)
                            for nj in range(2)
                        ]
                    attn_unit(t, qj, fillers)
            for st in range(4 * (QC - 1), 4 * QC):
                for nj in range(2):
                    emit_outproj(st, nj)

    nc.compile()
    return nc


_NC_CACHE = {}


def _get_nc():
    if "nc" not in _NC_CACHE:
        _NC_CACHE["nc"] = build()
    return _NC_CACHE["nc"]


B, S, E, H, DH = 4, 2048, 1024, 16, 64
GD = (H // 2) * DH  # 512 per-core head dims


def _in_maps(x, Wq, Wk, Wv, Wo):
    import ml_dtypes

    bf = ml_dtypes.bfloat16
    maps = []
    xt = [np.ascontiguousarray(x[b].T).astype(bf) for b in range(B)]
    for c in range(8):
        b, g = c // 2, c % 2
        sl = slice(g * GD, (g + 1) * GD)
        maps.append({
            "xt": xt[b],
            "wq": Wq[:, sl].astype(bf),
            "wk": Wk[:, sl].astype(bf),
            "wv": Wv[:, sl].astype(bf),
            "wo": Wo[sl, :].astype(bf),
        })
    return maps


def kernel(x, Wq, Wk, Wv, Wo):
    from concourse.bass_utils import run_bass_kernel_spmd

    x = np.asarray(x, dtype=np.float32)
    Wq = np.asarray(Wq, dtype=np.float32)
    Wk = np.asarray(Wk, dtype=np.float32)
    Wv = np.asarray(Wv, dtype=np.float32)
    Wo = np.asarray(Wo, dtype=np.float32)

    res = run_bass_kernel_spmd(
        _get_nc(), _in_maps(x, Wq, Wk, Wv, Wo), list(range(8))
    )

    out = np.empty((B, S, E), np.float32)
    for b in range(B):
        out[b] = res.results[2 * b]["out"] + res.results[2 * b + 1]["out"]
    return out


# revision 60
# speedup vs baseline: 1.0105x; 1.0105x over previous
"""Causal self-attention on 8 Trainium2 NeuronCores (Bass/Tile).

Problem: y = CausalSelfAttention(x; Wq, Wk, Wv, Wo) with
B=4, S=2048, E=1024, H=16 heads of 64, fp32 inputs/outputs.

Sharding (tensor-parallel x data-parallel): core c of 8 handles batch
b = c//2 and head-group g = c%2 (8 of 16 heads). Each core receives
x[b] [S, E], the head-group's columns of Wq/Wk/Wv [E, 512] and rows of
Wo [512, E], and produces a partial output projection [S, E]. The host
sums the two partials per batch.

Per-core dataflow (attention-path operands bf16, accumulation fp32):
  x arrives pre-transposed from the host (xt [E, S]) so every DMA is a
    plain contiguous 2D tile load on one queue, ordered by first use
    (the on-device XBAR transpose is a serial ~40us resource that
    otherwise gates the whole pipeline)
  qT/kT [512, S] = W.T @ x.T and v [S, 512] = x @ Wv, plain
    PSUM-accumulated matmul chains (kt-ordered so they pipeline with
    the arriving x tiles)
  per head-pair t, q-chunk of 512, k-tile of 128:
    ST [128, 2x512] = K @ Q.T   (two heads row-packed into disjoint
                                 64-row PE tile groups, concurrent)
    PT = exp(ST/8)              (one wide ACTIVATE per pair; causal mask
                                 = triangular-mask multiply on the
                                 diagonal subblock, both heads in one
                                 strided DVE op)
    AV [128, 512] += [V_h0|V_h1] col-packed PV pair (M=64+64, own rhs
                                 streams); SM [128, 512] += ones-matmul
                                 pair producing broadcast softmax sums
                                 (same col-packed shape, no extra pass
                                 for the bc broadcast)
  attT = AV * reciprocal(SM)    (two DVE ops per (pair, q-chunk))
  out = sum_t attT_t.T @ Wo_t accumulated over all 4 pairs in PSUM,
    evacuated once per output tile, DMA'd out.

Scheduling: the attention inner loop is balanced ~1:1 between the PE
(QK + PV + sums, ~1.0us/k-tile-pair) and the scalar engine's EXP
stream (~1.9us/pair, ~150us total), so projection work is diced into
8-matmul bursts and popped one-per-pair between the QK and PV groups
of each attention unit; the t=3 units interleave the output projection
the same way.  PSUM layout: QK double-buffer 4 banks, projection ring
2 banks (its own pool, so fillers never serialize behind the QK->EXP
pipeline), AV 1, SM 1.

No device collectives; the host slices inputs and sums partials.
"""

import numpy as np

import concourse.bass as bass
import concourse.mybir as mybir
from concourse import bacc
from concourse.tile import TileContext

FP = mybir.dt.float32
BF = mybir.dt.bfloat16
P = 128


def build(S=2048, E=1024, HPC=8, DH=64, NQ=512):
    GD = HPC * DH          # 512 head dims per core
    KT_E = E // P          # 8 contraction tiles over E
    ST_S = S // P          # 16 sequence tiles
    QC = S // NQ           # 4 q-chunks
    DT = GD // P           # 4 head pairs
    QSUB = NQ // P         # 4 k-tiles per q-chunk step

    assert DH == 64 and NQ % P == 0 and S % NQ == 0 and E % P == 0

    nc = bacc.Bacc(None, target_bir_lowering=False)
    xt_d = nc.dram_tensor("xt", [E, S], BF, kind="ExternalInput")
    wq_d = nc.dram_tensor("wq", [E, GD], BF, kind="ExternalInput")
    wk_d = nc.dram_tensor("wk", [E, GD], BF, kind="ExternalInput")
    wv_d = nc.dram_tensor("wv", [E, GD], BF, kind="ExternalInput")
    wo_d = nc.dram_tensor("wo", [GD, E], BF, kind="ExternalInput")
    out_d = nc.dram_tensor("out", [S, E], FP, kind="ExternalOutput")

    with TileContext(nc) as tc:
        with (
            tc.tile_pool(name="consts", bufs=1) as consts,
            tc.tile_pool(name="data", bufs=1) as data,
            tc.tile_pool(name="xT", bufs=1) as xT_pool,
            tc.tile_pool(name="wbuf", bufs=1) as wbuf,
            tc.tile_pool(name="pt_sb", bufs=8) as pt_pool,
            tc.tile_pool(name="rec_sb", bufs=2) as rec_pool,
            tc.tile_pool(name="po_sb", bufs=2) as posb_pool,
            tc.tile_pool(name="st_psum", bufs=2, space="PSUM") as st_pool,
            tc.tile_pool(name="pj_psum", bufs=2, space="PSUM") as pj_pool,
            tc.tile_pool(name="av_psum", bufs=1, space="PSUM") as av_pool,
            tc.tile_pool(name="sm_psum", bufs=1, space="PSUM") as sm_pool,
        ):
            # ---- constants -------------------------------------------------
            ones64 = consts.tile([P, 64], BF)
            nc.vector.memset(ones64[:], 1.0)
            # upper-triangular-inclusive multiplicative mask (valid k <= q),
            # replicated side by side for the two packed heads
            ut = consts.tile([P, P], BF)
            nc.gpsimd.memset(ut[:], 0.0)
            nc.gpsimd.affine_select(
                out=ut[:], in_=ut[:],
                compare_op=mybir.AluOpType.is_gt, fill=1.0,
                base=0, pattern=[[-1, P]], channel_multiplier=1,
            )
            ut2 = consts.tile([P, 2 * P], BF)
            nc.vector.tensor_copy(ut2[:, 0:P], ut[:])
            nc.vector.tensor_copy(ut2[:, P : 2 * P], ut[:])
            # preload the EXP table set while DMAs are in flight
            dummy_in = consts.tile([1, 1], FP)
            nc.vector.memset(dummy_in[:], 0.0)
            dummy_out = consts.tile([1, 1], BF)
            nc.scalar.activation(
                dummy_out[:], dummy_in[:], mybir.ActivationFunctionType.Exp,
                scale=1.0,
            )

            # ---- persistent SBUF data -------------------------------------
            kT = [data.tile([P, S], BF, tag=f"kT{t}", name=f"kT{t}") for t in range(DT)]
            qT = [data.tile([P, S], BF, tag=f"qT{t}", name=f"qT{t}") for t in range(DT)]
            v = [data.tile([P, GD], BF, tag=f"v{st}", name=f"v{st}") for st in range(ST_S)]
            attT = [data.tile([P, S], BF, tag=f"attT{t}", name=f"attT{t}") for t in range(DT)]
            xTc = [
                [xT_pool.tile([P, NQ], BF, tag=f"xT{et}_{sc}", name=f"xT{et}_{sc}")
                 for sc in range(QC)]
                for et in range(KT_E)
            ]
            wk_all = [wbuf.tile([P, KT_E * P], BF, tag=f"wk{mt}", name=f"wk{mt}") for mt in range(DT)]
            wq_all = [wbuf.tile([P, KT_E * P], BF, tag=f"wq{mt}", name=f"wq{mt}") for mt in range(DT)]
            wv_all = wbuf.tile([P, KT_E * GD], BF, tag="wv", name="wv")
            wo_all = wbuf.tile([P, DT * E], BF, tag="wo", name="wo")

            # One serial DMA queue, transfers ordered by first use.  x
            # arrives pre-transposed from the host, so every load is a
            # plain contiguous 2D tile descriptor (no XBAR serialization).
            def emit_xt(sc):
                for et in range(KT_E):
                    nc.sync.dma_start(
                        out=xTc[et][sc][:],
                        in_=xt_d[et * P : (et + 1) * P, sc * NQ : (sc + 1) * NQ],
                    )

            def emit_wkq(mt):
                for w_d, all_t in ((wk_d, wk_all), (wq_d, wq_all)):
                    for kt in range(KT_E):
                        nc.sync.dma_start(
                            out=all_t[mt][:, kt * P : (kt + 1) * P],
                            in_=w_d[kt * P : (kt + 1) * P, mt * P : (mt + 1) * P],
                        )

            # chunk0 x / k0 / q0 / wv interleaved per-kt so every chunk-0
            # projection chain (k, q, and v) starts ~1us in and pipelines
            # kt-by-kt with the DMA stream
            for kt in range(KT_E):
                nc.sync.dma_start(
                    out=xTc[kt][0][:], in_=xt_d[kt * P : (kt + 1) * P, 0:NQ]
                )
                nc.sync.dma_start(
                    out=wk_all[0][:, kt * P : (kt + 1) * P],
                    in_=wk_d[kt * P : (kt + 1) * P, 0:P],
                )
                nc.sync.dma_start(
                    out=wq_all[0][:, kt * P : (kt + 1) * P],
                    in_=wq_d[kt * P : (kt + 1) * P, 0:P],
                )
                nc.sync.dma_start(
                    out=wv_all[:, kt * GD : (kt + 1) * GD],
                    in_=wv_d[kt * P : (kt + 1) * P, :],
                )
            emit_xt(1)
            emit_wkq(1)
            emit_xt(2)
            emit_wkq(2)
            for t in range(DT):
                nc.sync.dma_start(
                    out=wo_all[:, t * E : (t + 1) * E],
                    in_=wo_d[t * P : (t + 1) * P, :],
                )
            emit_xt(3)
            emit_wkq(3)

            def kq_operands(mt, nsc, which, kt):
                w_all = (wk_all, wq_all)[which][mt]
                return w_all[:, kt * P : (kt + 1) * P], xTc[kt][nsc][:]

            def v_operands(st_i, kt):
                sc, r = divmod(st_i * P, NQ)
                return (xTc[kt][sc][:, r : r + P],
                        wv_all[:, kt * GD : (kt + 1) * GD])

            def proj_halves(operands, cast_out, name):
                """A projection chain as two ~1us filler closures: the first
                opens the PSUM accumulation (kt 0-3), the second finishes it
                and casts.  Halves are always consumed in order (FIFO)."""
                box = {}
                half_kt = KT_E // 2

                def first():
                    ps = pj_pool.tile([P, NQ], FP, tag="pj", name=name)
                    box["ps"] = ps
                    for kt in range(half_kt):
                        lhsT, rhs = operands(kt)
                        nc.tensor.matmul(ps[:, 0:NQ], lhsT=lhsT, rhs=rhs,
                                         start=(kt == 0), stop=False)

                def second():
                    ps = box["ps"]
                    for kt in range(half_kt, KT_E):
                        lhsT, rhs = operands(kt)
                        nc.tensor.matmul(ps[:, 0:NQ], lhsT=lhsT, rhs=rhs,
                                         start=False, stop=(kt == KT_E - 1))
                    nc.vector.tensor_copy(cast_out, ps[:, 0:NQ])

                return [first, second]

            def kq_halves(mt, nsc, which):
                dstT = (kT, qT)[which]
                return proj_halves(
                    lambda kt, m=mt, n=nsc, w=which: kq_operands(m, n, w, kt),
                    dstT[mt][:, nsc * NQ : (nsc + 1) * NQ],
                    f"pj{which}_{mt}_{nsc}",
                )

            def v_halves(st_i):
                return proj_halves(
                    lambda kt, s=st_i: v_operands(s, kt),
                    v[st_i][:], f"pv{st_i}",
                )

            def emit_proj_kq(mt, nsc, which):
                for f in kq_halves(mt, nsc, which):
                    f()

            def emit_proj_v(st_i):
                for f in v_halves(st_i):
                    f()

            def emit_outproj(st, nj):
                po = pj_pool.tile([P, NQ], FP, tag="pj", name=f"po{st}_{nj}")
                for t in range(DT):
                    nc.tensor.matmul(
                        po[:, 0:NQ],
                        lhsT=attT[t][:, st * P : (st + 1) * P],
                        rhs=wo_all[:, t * E + nj * NQ : t * E + (nj + 1) * NQ],
                        start=(t == 0), stop=(t == DT - 1),
                    )
                posb = posb_pool.tile([P, NQ], FP, tag="posb")
                nc.vector.tensor_copy(posb[:], po[:])
                nc.sync.dma_start(
                    out=out_d[st * P : (st + 1) * P, nj * NQ : (nj + 1) * NQ],
                    in_=posb[:],
                )

            def attn_unit(t, qj, fillers):
                n_tiles = QSUB * qj + QSUB
                kmax = n_tiles - 1
                av = av_pool.tile([P, NQ], FP, tag="av", name=f"av{t}_{qj}")
                sm = sm_pool.tile([P, NQ], FP, tag="sm", name=f"sm{t}_{qj}")

                def qk(ki):
                    stp = st_pool.tile([P, 2 * NQ], FP, tag="st")
                    d = ki - QSUB * qj
                    off = P * d if d > 0 else 0
                    for half in range(2):
                        pr = 64 * half
                        nc.tensor.matmul(
                            stp[:, half * NQ + off : (half + 1) * NQ],
                            lhsT=kT[t][pr : pr + 64, ki * P : (ki + 1) * P],
                            rhs=qT[t][pr : pr + 64, qj * NQ + off : (qj + 1) * NQ],
                            start=True, stop=True,
                        )
                    return stp, off, d

                def exp_mask(stp, off, d):
                    pt = pt_pool.tile([P, 2 * NQ], BF, tag="pt")
                    if off == 0:
                        nc.scalar.activation(
                            pt[:, 0 : 2 * NQ], stp[:, 0 : 2 * NQ],
                            mybir.ActivationFunctionType.Exp, scale=0.125,
                        )
                    else:
                        # one ACTIVATE over both heads' valid spans via a
                        # strided AP; dead cols are never read downstream
                        pt2 = pt.rearrange("p (k c) -> p k c", c=NQ)
                        st2 = stp.rearrange("p (k c) -> p k c", c=NQ)
                        nc.scalar.activation(
                            pt2[:, :, off:NQ], st2[:, :, off:NQ],
                            mybir.ActivationFunctionType.Exp, scale=0.125,
                        )
                    if d >= 0:
                        # causal mask on the diagonal subblock, both heads in
                        # one strided op
                        pt3 = pt.rearrange("p (k c) -> p k c", c=NQ)
                        ut3 = ut2.rearrange("p (k c) -> p k c", c=P)
                        nc.vector.tensor_tensor(
                            pt3[:, :, off : off + P], pt3[:, :, off : off + P],
                            ut3[:], mybir.AluOpType.mult,
                        )
                    return pt

                def pv_sums(pt, off, ki):
                    st_f, sp_f = (ki == 0), (ki == kmax)
                    for half in range(2):
                        h = 2 * t + half
                        nc.tensor.matmul(
                            av[64 * half : 64 * half + 64, off:NQ],
                            lhsT=v[ki][:, h * DH : h * DH + DH],
                            rhs=pt[:, half * NQ + off : (half + 1) * NQ],
                            start=st_f, stop=sp_f, skip_group_check=True,
                        )
                        nc.tensor.matmul(
                            sm[64 * half : 64 * half + 64, off:NQ],
                            lhsT=ones64[:],
                            rhs=pt[:, half * NQ + off : (half + 1) * NQ],
                            start=st_f, stop=sp_f, skip_group_check=True,
                        )

                # ki-pairs keep same-shape instruction streaks on the PE;
                # one projection burst rides between consecutive pairs
                for kp in range(n_tiles // 2):
                    kis = (2 * kp, 2 * kp + 1)
                    sts = [qk(ki) for ki in kis]
                    pts = [exp_mask(stp, off, d) for stp, off, d in sts]
                    # filler rides between QK and PV in the PE stream, so
                    # the PE chews it while the scalar engine runs the EXPs
                    if fillers:
                        fillers.pop(0)()
                    for ki, pt, (stp, off, d) in zip(kis, pts, sts):
                        pv_sums(pt, off, ki)

                while fillers:
                    fillers.pop(0)()

                rec = rec_pool.tile([P, NQ], FP, tag="rec")
                nc.vector.reciprocal_approx_fast(rec[:], sm[:])
                nc.vector.tensor_tensor(
                    attT[t][:, qj * NQ : (qj + 1) * NQ], av[:], rec[:],
                    mybir.AluOpType.mult,
                )

            # ---- main pipeline --------------------------------------------
            def chunk_group(nsc):
                g = [lambda n=nsc: emit_proj_kq(0, n, 0),
                     lambda n=nsc: emit_proj_kq(0, n, 1)]
                g += [lambda s=st_i: emit_proj_v(s)
                      for st_i in range(4 * nsc, 4 * nsc + 4)]
                return g

            # chunk 0 must fully precede the first attention unit
            for f in chunk_group(0):
                f()

            pending = []  # (needed_before_key, closure), key = (t, qj)
            for nsc in range(1, QC):
                pending += [((0, nsc), f) for f in chunk_group(nsc)]
            for mt in range(1, DT):
                for nsc in range(QC):
                    for which in (0, 1):
                        pending.append((
                            (mt, 0 if mt > 1 else nsc),
                            lambda m=mt, n=nsc, w=which: emit_proj_kq(m, n, w),
                        ))

            for t in range(DT):
                for qj in range(QC):
                    while pending and pending[0][0] <= (t, qj):
                        pending.pop(0)[1]()
                    fillers = []
                    if pending:
                        take = min(len(pending), (QSUB * qj + QSUB) >> 1, 4)
                        fillers = [f for _, f in pending[:take]]
                        del pending[:take]
                    if t == DT - 1 and qj > 0:
                        # last pair: interleave the output projection of the
                        # previous q-chunk's s-tiles
                        fillers += [
                            lambda s=st, n=nj: emit_outproj(s, n)
                            for st in range(4 * (qj - 1), 4 * qj)
                            for nj in range(2)
                        ]
                    attn_unit(t, qj, fillers)
            for st in range(4 * (QC - 1), 4 * QC):
                for nj in range(2):
                    emit_outproj(st, nj)

    nc.compile()
    return nc


_NC_CACHE = {}


def _get_nc():
    if "nc" not in _NC_CACHE:
        _NC_CACHE["nc"] = build()
    return _NC_CACHE["nc"]


B, S, E, H, DH = 4, 2048, 1024, 16, 64
GD = (H // 2) * DH  # 512 per-core head dims


def _in_maps(x, Wq, Wk, Wv, Wo):
    import ml_dtypes

    bf = ml_dtypes.bfloat16
    maps = []
    xt = [np.ascontiguousarray(x[b].T).astype(bf) for b in range(B)]
    for c in range(8):
        b, g = c // 2, c % 2
        sl = slice(g * GD, (g + 1) * GD)
        maps.append({
            "xt": xt[b],
            "wq": Wq[:, sl].astype(bf),
            "wk": Wk[:, sl].astype(bf),
            "wv": Wv[:, sl].astype(bf),
            "wo": Wo[sl, :].astype(bf),
        })
    return maps


def kernel(x, Wq, Wk, Wv, Wo):
    from concourse.bass_utils import run_bass_kernel_spmd

    x = np.asarray(x, dtype=np.float32)
    Wq = np.asarray(Wq, dtype=np.float32)
    Wk = np.asarray(Wk, dtype=np.float32)
    Wv = np.asarray(Wv, dtype=np.float32)
    Wo = np.asarray(Wo, dtype=np.float32)

    res = run_bass_kernel_spmd(
        _get_nc(), _in_maps(x, Wq, Wk, Wv, Wo), list(range(8))
    )

    out = np.empty((B, S, E), np.float32)
    for b in range(B):
        out[b] = res.results[2 * b]["out"] + res.results[2 * b + 1]["out"]
    return out


# revision 61
# speedup vs baseline: 1.0142x; 1.0036x over previous
"""Causal self-attention on 8 Trainium2 NeuronCores (Bass/Tile).

Problem: y = CausalSelfAttention(x; Wq, Wk, Wv, Wo) with
B=4, S=2048, E=1024, H=16 heads of 64, fp32 inputs/outputs.

Sharding (tensor-parallel x data-parallel): core c of 8 handles batch
b = c//2 and head-group g = c%2 (8 of 16 heads). Each core receives
x[b] [S, E], the head-group's columns of Wq/Wk/Wv [E, 512] and rows of
Wo [512, E], and produces a partial output projection [S, E]. The host
sums the two partials per batch.

Per-core dataflow (attention-path operands bf16, accumulation fp32):
  x arrives pre-transposed from the host (xt [E, S]) so every DMA is a
    plain contiguous 2D tile load on one queue, ordered by first use
    (the on-device XBAR transpose is a serial ~40us resource that
    otherwise gates the whole pipeline)
  qT/kT [512, S] = W.T @ x.T and v [S, 512] = x @ Wv, plain
    PSUM-accumulated matmul chains (kt-ordered so they pipeline with
    the arriving x tiles)
  per head-pair t, q-chunk of 512, k-tile of 128:
    ST [128, 2x512] = K @ Q.T   (two heads row-packed into disjoint
                                 64-row PE tile groups, concurrent)
    PT = exp(ST/8)              (one wide ACTIVATE per pair; causal mask
                                 = triangular-mask multiply on the
                                 diagonal subblock, both heads in one
                                 strided DVE op)
    AV [128, 512] += [V_h0|V_h1] col-packed PV pair (M=64+64, own rhs
                                 streams); SM [128, 512] += ones-matmul
                                 pair producing broadcast softmax sums
                                 (same col-packed shape, no extra pass
                                 for the bc broadcast)
  attT = AV * reciprocal(SM)    (two DVE ops per (pair, q-chunk))
  out = sum_t attT_t.T @ Wo_t accumulated over all 4 pairs in PSUM,
    evacuated once per output tile, DMA'd out.

Scheduling: the attention inner loop is balanced ~1:1 between the PE
(QK + PV + sums, ~1.0us/k-tile-pair) and the scalar engine's EXP
stream (~1.9us/pair, ~150us total), so projection work is diced into
8-matmul bursts and popped one-per-pair between the QK and PV groups
of each attention unit; the t=3 units interleave the output projection
the same way.  PSUM layout: QK double-buffer 4 banks, projection ring
2 banks (its own pool, so fillers never serialize behind the QK->EXP
pipeline), AV 1, SM 1.

No device collectives; the host slices inputs and sums partials.
"""

import numpy as np

import concourse.bass as bass
import concourse.mybir as mybir
from concourse import bacc
from concourse.tile import TileContext

FP = mybir.dt.float32
BF = mybir.dt.bfloat16
P = 128


def build(S=2048, E=1024, HPC=8, DH=64, NQ=512):
    GD = HPC * DH          # 512 head dims per core
    KT_E = E // P          # 8 contraction tiles over E
    ST_S = S // P          # 16 sequence tiles
    QC = S // NQ           # 4 q-chunks
    DT = GD // P           # 4 head pairs
    QSUB = NQ // P         # 4 k-tiles per q-chunk step

    assert DH == 64 and NQ % P == 0 and S % NQ == 0 and E % P == 0

    nc = bacc.Bacc(None, target_bir_lowering=False)
    xt_d = nc.dram_tensor("xt", [E, S], BF, kind="ExternalInput")
    wq_d = nc.dram_tensor("wq", [E, GD], BF, kind="ExternalInput")
    wk_d = nc.dram_tensor("wk", [E, GD], BF, kind="ExternalInput")
    wv_d = nc.dram_tensor("wv", [E, GD], BF, kind="ExternalInput")
    wo_d = nc.dram_tensor("wo", [GD, E], BF, kind="ExternalInput")
    out_d = nc.dram_tensor("out", [S, E], FP, kind="ExternalOutput")

    with TileContext(nc) as tc:
        with (
            tc.tile_pool(name="consts", bufs=1) as consts,
            tc.tile_pool(name="data", bufs=1) as data,
            tc.tile_pool(name="xT", bufs=1) as xT_pool,
            tc.tile_pool(name="wbuf", bufs=1) as wbuf,
            tc.tile_pool(name="pt_sb", bufs=8) as pt_pool,
            tc.tile_pool(name="rec_sb", bufs=2) as rec_pool,
            tc.tile_pool(name="po_sb", bufs=2) as posb_pool,
            tc.tile_pool(name="st_psum", bufs=2, space="PSUM") as st_pool,
            tc.tile_pool(name="pj_psum", bufs=2, space="PSUM") as pj_pool,
            tc.tile_pool(name="av_psum", bufs=1, space="PSUM") as av_pool,
            tc.tile_pool(name="sm_psum", bufs=1, space="PSUM") as sm_pool,
        ):
            # ---- constants -------------------------------------------------
            ones64 = consts.tile([P, 64], BF)
            nc.vector.memset(ones64[:], 1.0)
            # upper-triangular-inclusive multiplicative mask (valid k <= q),
            # replicated side by side for the two packed heads
            ut = consts.tile([P, P], BF)
            nc.gpsimd.memset(ut[:], 0.0)
            nc.gpsimd.affine_select(
                out=ut[:], in_=ut[:],
                compare_op=mybir.AluOpType.is_gt, fill=1.0,
                base=0, pattern=[[-1, P]], channel_multiplier=1,
            )
            ut2 = consts.tile([P, 2 * P], BF)
            nc.vector.tensor_copy(ut2[:, 0:P], ut[:])
            nc.vector.tensor_copy(ut2[:, P : 2 * P], ut[:])
            # preload the EXP table set while DMAs are in flight
            dummy_in = consts.tile([1, 1], FP)
            nc.vector.memset(dummy_in[:], 0.0)
            dummy_out = consts.tile([1, 1], BF)
            nc.scalar.activation(
                dummy_out[:], dummy_in[:], mybir.ActivationFunctionType.Exp,
                scale=1.0,
            )

            # ---- persistent SBUF data -------------------------------------
            kT = [data.tile([P, S], BF, tag=f"kT{t}", name=f"kT{t}") for t in range(DT)]
            qT = [data.tile([P, S], BF, tag=f"qT{t}", name=f"qT{t}") for t in range(DT)]
            v = [data.tile([P, GD], BF, tag=f"v{st}", name=f"v{st}") for st in range(ST_S)]
            attT = [data.tile([P, S], BF, tag=f"attT{t}", name=f"attT{t}") for t in range(DT)]
            xTc = [
                [xT_pool.tile([P, NQ], BF, tag=f"xT{et}_{sc}", name=f"xT{et}_{sc}")
                 for sc in range(QC)]
                for et in range(KT_E)
            ]
            wk_all = [wbuf.tile([P, KT_E * P], BF, tag=f"wk{mt}", name=f"wk{mt}") for mt in range(DT)]
            wq_all = [wbuf.tile([P, KT_E * P], BF, tag=f"wq{mt}", name=f"wq{mt}") for mt in range(DT)]
            wv_all = wbuf.tile([P, KT_E * GD], BF, tag="wv", name="wv")
            wo_all = wbuf.tile([P, DT * E], BF, tag="wo", name="wo")

            # One serial DMA queue, transfers ordered by first use.  x
            # arrives pre-transposed from the host, so every load is a
            # plain contiguous 2D tile descriptor (no XBAR serialization).
            def emit_xt(sc):
                for et in range(KT_E):
                    nc.sync.dma_start(
                        out=xTc[et][sc][:],
                        in_=xt_d[et * P : (et + 1) * P, sc * NQ : (sc + 1) * NQ],
                    )

            def emit_wkq(mt):
                for w_d, all_t in ((wk_d, wk_all), (wq_d, wq_all)):
                    for kt in range(KT_E):
                        nc.sync.dma_start(
                            out=all_t[mt][:, kt * P : (kt + 1) * P],
                            in_=w_d[kt * P : (kt + 1) * P, mt * P : (mt + 1) * P],
                        )

            # chunk0 x / k0 / q0 / wv interleaved per-kt so every chunk-0
            # projection chain (k, q, and v) starts ~1us in and pipelines
            # kt-by-kt with the DMA stream
            for kt in range(KT_E):
                nc.sync.dma_start(
                    out=xTc[kt][0][:], in_=xt_d[kt * P : (kt + 1) * P, 0:NQ]
                )
                nc.sync.dma_start(
                    out=wk_all[0][:, kt * P : (kt + 1) * P],
                    in_=wk_d[kt * P : (kt + 1) * P, 0:P],
                )
                nc.sync.dma_start(
                    out=wq_all[0][:, kt * P : (kt + 1) * P],
                    in_=wq_d[kt * P : (kt + 1) * P, 0:P],
                )
                nc.sync.dma_start(
                    out=wv_all[:, kt * GD : (kt + 1) * GD],
                    in_=wv_d[kt * P : (kt + 1) * P, :],
                )
            emit_xt(1)
            emit_wkq(1)
            emit_xt(2)
            emit_wkq(2)
            for t in range(DT):
                nc.sync.dma_start(
                    out=wo_all[:, t * E : (t + 1) * E],
                    in_=wo_d[t * P : (t + 1) * P, :],
                )
            emit_xt(3)
            emit_wkq(3)

            def kq_operands(mt, nsc, which, kt):
                w_all = (wk_all, wq_all)[which][mt]
                return w_all[:, kt * P : (kt + 1) * P], xTc[kt][nsc][:]

            def v_operands(st_i, kt):
                sc, r = divmod(st_i * P, NQ)
                return (xTc[kt][sc][:, r : r + P],
                        wv_all[:, kt * GD : (kt + 1) * GD])

            def proj_halves(operands, cast_out, name):
                """A projection chain as two ~1us filler closures: the first
                opens the PSUM accumulation (kt 0-3), the second finishes it
                and casts.  Halves are always consumed in order (FIFO)."""
                box = {}
                half_kt = KT_E // 2

                def first():
                    ps = pj_pool.tile([P, NQ], FP, tag="pj", name=name)
                    box["ps"] = ps
                    for kt in range(half_kt):
                        lhsT, rhs = operands(kt)
                        nc.tensor.matmul(ps[:, 0:NQ], lhsT=lhsT, rhs=rhs,
                                         start=(kt == 0), stop=False)

                def second():
                    ps = box["ps"]
                    for kt in range(half_kt, KT_E):
                        lhsT, rhs = operands(kt)
                        nc.tensor.matmul(ps[:, 0:NQ], lhsT=lhsT, rhs=rhs,
                                         start=False, stop=(kt == KT_E - 1))
                    nc.vector.tensor_copy(cast_out, ps[:, 0:NQ])

                return [first, second]

            def kq_halves(mt, nsc, which):
                dstT = (kT, qT)[which]
                return proj_halves(
                    lambda kt, m=mt, n=nsc, w=which: kq_operands(m, n, w, kt),
                    dstT[mt][:, nsc * NQ : (nsc + 1) * NQ],
                    f"pj{which}_{mt}_{nsc}",
                )

            def v_halves(st_i):
                return proj_halves(
                    lambda kt, s=st_i: v_operands(s, kt),
                    v[st_i][:], f"pv{st_i}",
                )

            def emit_proj_kq(mt, nsc, which):
                for f in kq_halves(mt, nsc, which):
                    f()

            def emit_proj_v(st_i):
                for f in v_halves(st_i):
                    f()

            def emit_outproj(st, nj):
                po = pj_pool.tile([P, NQ], FP, tag="pj", name=f"po{st}_{nj}")
                for t in range(DT):
                    nc.tensor.matmul(
                        po[:, 0:NQ],
                        lhsT=attT[t][:, st * P : (st + 1) * P],
                        rhs=wo_all[:, t * E + nj * NQ : t * E + (nj + 1) * NQ],
                        start=(t == 0), stop=(t == DT - 1),
                    )
                posb = posb_pool.tile([P, NQ], FP, tag="posb")
                nc.vector.tensor_copy(posb[:], po[:])
                nc.sync.dma_start(
                    out=out_d[st * P : (st + 1) * P, nj * NQ : (nj + 1) * NQ],
                    in_=posb[:],
                )

            def attn_unit(t, qj, fillers):
                n_tiles = QSUB * qj + QSUB
                kmax = n_tiles - 1
                av = av_pool.tile([P, NQ], FP, tag="av", name=f"av{t}_{qj}")
                sm = sm_pool.tile([P, NQ], FP, tag="sm", name=f"sm{t}_{qj}")

                def qk(ki):
                    stp = st_pool.tile([P, 2 * NQ], FP, tag="st")
                    d = ki - QSUB * qj
                    off = P * d if d > 0 else 0
                    for half in range(2):
                        pr = 64 * half
                        nc.tensor.matmul(
                            stp[:, half * NQ + off : (half + 1) * NQ],
                            lhsT=kT[t][pr : pr + 64, ki * P : (ki + 1) * P],
                            rhs=qT[t][pr : pr + 64, qj * NQ + off : (qj + 1) * NQ],
                            start=True, stop=True,
                        )
                    return stp, off, d

                def exp_mask(stp, off, d):
                    pt = pt_pool.tile([P, 2 * NQ], BF, tag="pt")
                    if off == 0:
                        nc.scalar.activation(
                            pt[:, 0 : 2 * NQ], stp[:, 0 : 2 * NQ],
                            mybir.ActivationFunctionType.Exp, scale=0.125,
                        )
                    else:
                        # one ACTIVATE over both heads' valid spans via a
                        # strided AP; dead cols are never read downstream
                        pt2 = pt.rearrange("p (k c) -> p k c", c=NQ)
                        st2 = stp.rearrange("p (k c) -> p k c", c=NQ)
                        nc.scalar.activation(
                            pt2[:, :, off:NQ], st2[:, :, off:NQ],
                            mybir.ActivationFunctionType.Exp, scale=0.125,
                        )
                    if d >= 0:
                        # causal mask on the diagonal subblock, both heads in
                        # one strided op
                        pt3 = pt.rearrange("p (k c) -> p k c", c=NQ)
                        ut3 = ut2.rearrange("p (k c) -> p k c", c=P)
                        nc.vector.tensor_tensor(
                            pt3[:, :, off : off + P], pt3[:, :, off : off + P],
                            ut3[:], mybir.AluOpType.mult,
                        )
                    return pt

                def pv_sums(pt, off, ki):
                    # sums first: the finalize reciprocal depends on the last
                    # sums matmul, so emitting sums ahead of PV lets it
                    # overlap the unit's final PV streams
                    st_f, sp_f = (ki == 0), (ki == kmax)
                    for half in range(2):
                        nc.tensor.matmul(
                            sm[64 * half : 64 * half + 64, off:NQ],
                            lhsT=ones64[:],
                            rhs=pt[:, half * NQ + off : (half + 1) * NQ],
                            start=st_f, stop=sp_f, skip_group_check=True,
                        )
                    for half in range(2):
                        h = 2 * t + half
                        nc.tensor.matmul(
                            av[64 * half : 64 * half + 64, off:NQ],
                            lhsT=v[ki][:, h * DH : h * DH + DH],
                            rhs=pt[:, half * NQ + off : (half + 1) * NQ],
                            start=st_f, stop=sp_f, skip_group_check=True,
                        )

                # ki-pairs keep same-shape instruction streaks on the PE;
                # one projection burst rides between consecutive pairs
                for kp in range(n_tiles // 2):
                    kis = (2 * kp, 2 * kp + 1)
                    sts = [qk(ki) for ki in kis]
                    pts = [exp_mask(stp, off, d) for stp, off, d in sts]
                    # filler rides between QK and PV in the PE stream, so
                    # the PE chews it while the scalar engine runs the EXPs
                    if fillers:
                        fillers.pop(0)()
                    for ki, pt, (stp, off, d) in zip(kis, pts, sts):
                        pv_sums(pt, off, ki)

                while fillers:
                    fillers.pop(0)()

                rec = rec_pool.tile([P, NQ], FP, tag="rec")
                nc.vector.reciprocal_approx_fast(rec[:], sm[:])
                nc.vector.tensor_tensor(
                    attT[t][:, qj * NQ : (qj + 1) * NQ], av[:], rec[:],
                    mybir.AluOpType.mult,
                )

            # ---- main pipeline --------------------------------------------
            def chunk_group(nsc):
                g = [lambda n=nsc: emit_proj_kq(0, n, 0),
                     lambda n=nsc: emit_proj_kq(0, n, 1)]
                g += [lambda s=st_i: emit_proj_v(s)
                      for st_i in range(4 * nsc, 4 * nsc + 4)]
                return g

            # chunk 0 must fully precede the first attention unit
            for f in chunk_group(0):
                f()

            pending = []  # (needed_before_key, closure), key = (t, qj)
            for nsc in range(1, QC):
                pending += [((0, nsc), f) for f in chunk_group(nsc)]
            for mt in range(1, DT):
                for nsc in range(QC):
                    for which in (0, 1):
                        pending.append((
                            (mt, 0 if mt > 1 else nsc),
                            lambda m=mt, n=nsc, w=which: emit_proj_kq(m, n, w),
                        ))

            for t in range(DT):
                for qj in range(QC):
                    while pending and pending[0][0] <= (t, qj):
                        pending.pop(0)[1]()
                    fillers = []
                    if pending:
                        take = min(len(pending), (QSUB * qj + QSUB) >> 1, 4)
                        fillers = [f for _, f in pending[:take]]
                        del pending[:take]
                    if t == DT - 1 and qj > 0:
                        # last pair: interleave the output projection of the
                        # previous q-chunk's s-tiles
                        fillers += [
                            lambda s=st, n=nj: emit_outproj(s, n)
                            for st in range(4 * (qj - 1), 4 * qj)
                            for nj in range(2)
                        ]
                    attn_unit(t, qj, fillers)
            for st in range(4 * (QC - 1), 4 * QC):
                for nj in range(2):
                    emit_outproj(st, nj)

    nc.compile()
    return nc


_NC_CACHE = {}


def _get_nc():
    if "nc" not in _NC_CACHE:
        _NC_CACHE["nc"] = build()
    return _NC_CACHE["nc"]


B, S, E, H, DH = 4, 2048, 1024, 16, 64
GD = (H // 2) * DH  # 512 per-core head dims


def _in_maps(x, Wq, Wk, Wv, Wo):
    import ml_dtypes

    bf = ml_dtypes.bfloat16
    maps = []
    xt = [np.ascontiguousarray(x[b].T).astype(bf) for b in range(B)]
    for c in range(8):
        b, g = c // 2, c % 2
        sl = slice(g * GD, (g + 1) * GD)
        maps.append({
            "xt": xt[b],
            "wq": Wq[:, sl].astype(bf),
            "wk": Wk[:, sl].astype(bf),
            "wv": Wv[:, sl].astype(bf),
            "wo": Wo[sl, :].astype(bf),
        })
    return maps


def kernel(x, Wq, Wk, Wv, Wo):
    from concourse.bass_utils import run_bass_kernel_spmd

    x = np.asarray(x, dtype=np.float32)
    Wq = np.asarray(Wq, dtype=np.float32)
    Wk = np.asarray(Wk, dtype=np.float32)
    Wv = np.asarray(Wv, dtype=np.float32)
    Wo = np.asarray(Wo, dtype=np.float32)

    res = run_bass_kernel_spmd(
        _get_nc(), _in_maps(x, Wq, Wk, Wv, Wo), list(range(8))
    )

    out = np.empty((B, S, E), np.float32)
    for b in range(B):
        out[b] = res.results[2 * b]["out"] + res.results[2 * b + 1]["out"]
    return out
